# revision 17
# baseline (speedup 1.0000x reference)
"""CoAttention kernel for 8 TRN2 NeuronCores (data-parallel over batch).

Math (per batch b):
  fm = maps.reshape(B, 196, 2048)
  ml = fm @ Wm + bm                       (196, 512)
  hl = hiddens @ Wh + bh                  (64, 512)
  scores[t,s] = sum_a Wr[a]*relu(ml[s,a]+hl[t,a]) + br
  softmap = softmax(scores, axis=s)
  ctx = softmap @ fm                      (64, 2048)
  co_att = (ctx @ Wc + bc) * hiddens      (64, 768)
  returns (co_att, softmap)

Key tricks:
  * br cancels in the softmax -> dropped entirely.
  * |Wr| is folded into Wm/bm/Wh/bh host-side, so
    Wr[a]*relu(ml+hl) == sign(Wr[a]) * relu(ml''+hl'') elementwise, and the
    a-reduction becomes a single ones/sign-weighted matmul (lhsT = sign vec).
  * rect = relu(ml'' + hl''_t) is one fused DVE tensor_scalar (add, max 0)
    or one ACT activation(Relu, bias) per (t, a-chunk); both engines share it.
  * The a-reduction runs on the PE with 128x32 column tiling: 4 independent
    column tiles each own 16 of the 64 t's (4x matmul throughput at M=1).
  * bm/bh/bc are folded in as K=1 rank-1 bias matmuls (ones x bias_row).
"""

import os
import sys

for _p in ("/opt/trn_rl_repo",):
    if _p not in sys.path:
        sys.path.insert(0, _p)

from contextlib import ExitStack

import ml_dtypes
import numpy as np

import concourse.bass as bass
import concourse.mybir as mybir
import concourse.tile as tile
from concourse import bacc
from concourse.bass_utils import run_bass_kernel_spmd
from concourse.masks import make_identity
from concourse.tile import add_dep_helper

BF16 = mybir.dt.bfloat16
F32 = mybir.dt.float32
AF = mybir.ActivationFunctionType
ALU = mybir.AluOpType
NPBF16 = ml_dtypes.bfloat16

B, S, D, H, A, T = 16, 196, 2048, 768, 512, 64
NCORES = 8
BPC = B // NCORES  # batches per core
NJ = A // 128      # 4 a-chunks
KD = D // 128      # 16
KH = H // 128      # 6
SC = S // 2        # 98, s-halves for ctx contraction
SP = 208           # maps rows padded to a multiple of 16 for the xbar transpose

USE_COL_TILING = True
P1_ONLY = bool(os.environ.get("K_P1_ONLY"))
ACT_RECT_FRAC = 4   # every 4th t's rect ops go to ScalarE, rest on VectorE


def _emit(ctx: ExitStack, tc: "tile.TileContext", io: dict):
    nc = tc.nc
    wp = ctx.enter_context(tc.tile_pool(name="wts", bufs=1))
    dp = ctx.enter_context(tc.tile_pool(name="data", bufs=2))
    mp = ctx.enter_context(tc.tile_pool(name="mlhl", bufs=2))
    rp = ctx.enter_context(tc.tile_pool(name="rect", bufs=8))
    sp = ctx.enter_context(tc.tile_pool(name="small", bufs=2))
    pm = ctx.enter_context(tc.tile_pool(name="psP1", bufs=1, space="PSUM"))
    psc = ctx.enter_context(tc.tile_pool(name="psSc", bufs=2, space="PSUM"))
    p3 = ctx.enter_context(tc.tile_pool(name="psP3", bufs=1, space="PSUM"))

    # ---- weights / constants (loaded once) ----
    wm_sb = wp.tile([128, KD, A], BF16)
    nc.sync.dma_start(wm_sb[:], io["wm"].rearrange("(c p) a -> p c a", p=128))
    wh_sb = wp.tile([128, KH, A], BF16)
    nc.sync.dma_start(wh_sb[:], io["wh"].rearrange("(c p) a -> p c a", p=128))
    wc_sb = wp.tile([128, KD, H], BF16)
    nc.sync.dma_start(wc_sb[:], io["wc"].rearrange("(c p) h -> p c h", p=128))
    bm_sb = wp.tile([1, A], BF16)
    nc.sync.dma_start(bm_sb[:], io["bm"][:])
    bh_sb = wp.tile([1, A], BF16)
    nc.sync.dma_start(bh_sb[:], io["bh"][:])
    wcb_sb = wp.tile([1, H], BF16)
    nc.sync.dma_start(wcb_sb[:], io["wcb"][:])
    # wsgm[p, j, t, m] = sign(Wr[128j+p]) if m == t else 0 — a one-hot sign
    # column so each t's reduction matmul writes psum row t of the packed
    # (64, S) scores tile while every matmul is a standard full-M write.
    wsgm_sb = wp.tile([128, NJ, T, T], BF16)
    nc.sync.dma_start(wsgm_sb[:], io["wsgm"][:])
    ones_sb = wp.tile([1, S], BF16)
    nc.vector.memset(ones_sb[:], 1.0)
    ident = wp.tile([128, 128], BF16)
    make_identity(nc, ident[:])

    pe_p1 = [[] for _ in range(BPC)]
    pe_p2 = [[] for _ in range(BPC)]
    pe_p3 = [[] for _ in range(BPC)]

    for b in range(BPC):
        # ---- P1: load batch data, compute ml'' and hl'' ----
        fmT = dp.tile([128, KD, SP], BF16, tag="fmT")
        for c in range(KD):
            # xbar transpose needs src rows % 16 == 0, hence the host-side
            # zero-pad of maps to SP=208 rows (the 4-row remainder fallback
            # produces garbage on hardware)
            nc.sync.dma_start_transpose(
                fmT[:, c, :], io["maps"][b, :, 128 * c : 128 * (c + 1)]
            )
        hidT = dp.tile([128, KH, T], BF16, tag="hidT")
        for c in range(KH):
            nc.sync.dma_start_transpose(
                hidT[:, c, :], io["hiddens"][b, :, 128 * c : 128 * (c + 1)]
            )
        fm_sb = dp.tile([SC, 2, D], BF16, tag="fm")
        nc.sync.dma_start(fm_sb[:, 0, :], io["maps"][b, 0:SC, :])
        nc.sync.dma_start(fm_sb[:, 1, :], io["maps"][b, SC:S, :])
        hid_sb = dp.tile([T, H], BF16, tag="hid")
        nc.sync.dma_start(hid_sb[:], io["hiddens"][b])

        # ml'' = (fm @ Wm'' + bm'')^T : NJ chunks of (128a, S), psum packed in pairs
        ml_sb = mp.tile([128, NJ, S], BF16, tag="ml")
        for jj in range(2):
            mlp = pm.tile([128, 2, S], F32, tag="mlp")
            for j2 in range(2):
                j = 2 * jj + j2
                for kc in range(KD):
                    pe_p1[b].append(nc.tensor.matmul(
                        mlp[:, j2, :],
                        lhsT=wm_sb[:, kc, 128 * j : 128 * (j + 1)],
                        rhs=fmT[:, kc, 0:S],
                        start=(kc == 0),
                        stop=False,
                    ))
                pe_p1[b].append(nc.tensor.matmul(
                    mlp[:, j2, :],
                    lhsT=bm_sb[:, 128 * j : 128 * (j + 1)],
                    rhs=ones_sb[:],
                    start=False,
                    stop=True,
                ))
            nc.vector.tensor_copy(ml_sb[:, 2 * jj : 2 * jj + 2, :], mlp[:])

        # hl'' = (hiddens @ Wh'' + bh'')^T : (128a, NJ, T) in one psum bank
        hlp = pm.tile([128, NJ, T], F32, tag="hlp")
        for j in range(NJ):
            for kc in range(KH):
                pe_p1[b].append(nc.tensor.matmul(
                    hlp[:, j, :],
                    lhsT=wh_sb[:, kc, 128 * j : 128 * (j + 1)],
                    rhs=hidT[:, kc, :],
                    start=(kc == 0),
                    stop=False,
                ))
            pe_p1[b].append(nc.tensor.matmul(
                hlp[:, j, :],
                lhsT=bh_sb[:, 128 * j : 128 * (j + 1)],
                rhs=ones_sb[:, 0:T],
                start=False,
                stop=True,
            ))
        hl_sb = mp.tile([128, NJ, T], F32, tag="hl")
        nc.vector.tensor_copy(hl_sb[:], hlp[:])

        if P1_ONLY:
            nc.sync.dma_start(io["dbg_ml"][b], ml_sb[:])
            nc.sync.dma_start(io["dbg_hl"][b], hl_sb[:])
            zf = sp.tile([T, H], F32, tag="coF")
            nc.vector.memset(zf[:], 0.0)
            nc.sync.dma_start(io["co_att"][b], zf[:])
            zs = sp.tile([T, S], F32, tag="smF")
            nc.vector.memset(zs[:], 0.0)
            nc.sync.dma_start(io["softmap"][b], zs[:])
            continue

        # ---- P2: rect + sign-weighted a-reduction -> scores ----
        # one accumulation chain into a packed (64, S) psum tile: matmul for
        # t adds its score into row t (one-hot lhsT column) and zeros elsewhere
        scp = psc.tile([T, 256], F32, tag="scores")  # 256-wide pad
        for t in range(T):
            rect = rp.tile([128, NJ, S], BF16, tag="rect")
            on_act = (t % ACT_RECT_FRAC) == ACT_RECT_FRAC - 1
            for j in range(NJ):
                if on_act:
                    nc.scalar.activation(
                        rect[:, j, :],
                        ml_sb[:, j, :],
                        AF.Relu,
                        bias=hl_sb[:, j, t : t + 1],
                    )
                else:
                    nc.vector.tensor_scalar(
                        rect[:, j, :],
                        ml_sb[:, j, :],
                        hl_sb[:, j, t : t + 1],
                        0.0,
                        op0=ALU.add,
                        op1=ALU.max,
                    )
            for j in range(NJ):
                pe_p2[b].append(nc.tensor.matmul(
                    scp[0:T, 0:S],
                    lhsT=wsgm_sb[:, j, t, :],
                    rhs=rect[:, j, :],
                    start=(t == 0 and j == 0),
                    stop=(t == T - 1 and j == NJ - 1),
                    skip_group_check=True,
                ))

        # ---- P3: softmax, softmap out, ctx, co_att ----
        exps = sp.tile([T, S], BF16, tag="exps")
        nc.scalar.activation(exps[:], scp[0:T, 0:S], AF.Exp)
        den = sp.tile([T, 1], F32, tag="den")
        nc.vector.reduce_sum(den[:], exps[:], axis=mybir.AxisListType.X)
        rec = sp.tile([T, 1], F32, tag="rec")
        nc.vector.reciprocal(rec[:], den[:])
        smF = sp.tile([T, S], F32, tag="smF")
        nc.vector.tensor_scalar(
            smF[:], exps[:], rec[:, 0:1], None, op0=ALU.mult
        )
        nc.sync.dma_start(io["softmap"][b], smF[:])
        # normalized bf16 softmap on 128 partitions for the PE transpose
        # (rows 64-127 are never written; the transpose result columns they
        # produce are simply not gathered)
        smN = sp.tile([128, S], BF16, tag="smN")
        nc.vector.tensor_scalar(
            smN[0:T, :], exps[:], rec[:, 0:1], None, op0=ALU.mult
        )

        # softmap^T (normalized, bf16) via two full-width PE transposes
        # (K=128 keeps the PE in plain 128x128 mode); valid t-columns sit at
        # the spread positions {32c+i} and are gathered in the psum->sbuf copy.
        smT_ps = p3.tile([SC, 2, 128], BF16, tag="smTp")
        for k in range(2):
            pe_p3[b].append(nc.tensor.transpose(
                smT_ps[:, k, :],
                smN[:, SC * k : SC * (k + 1)],
                ident[:],
            ))
        smT = sp.tile([SC, 2, T], BF16, tag="smT")
        nc.vector.tensor_copy(smT[:], smT_ps[:, :, 0:T])

        # ctx^T: (d, t) in two psum banks of 8 d-chunks each
        cxT = sp.tile([128, KD, T], BF16, tag="cxT")
        for hh in range(2):
            cxp = p3.tile([128, 8, T], F32, tag="cxp")
            for k8 in range(8):
                k = 8 * hh + k8
                for k2 in range(2):
                    pe_p3[b].append(nc.tensor.matmul(
                        cxp[:, k8, :],
                        lhsT=fm_sb[:, k2, 128 * k : 128 * (k + 1)],
                        rhs=smT[:, k2, :],
                        start=(k2 == 0),
                        stop=(k2 == 1),
                    ))
            nc.vector.tensor_copy(cxT[:, 8 * hh : 8 * hh + 8, :], cxp[:])

        # co_att = (ctx @ Wc + bc) * hiddens
        cop0 = p3.tile([T, 512], F32, tag="cop0")
        cop1 = p3.tile([T, H - 512], F32, tag="cop1")
        for k in range(KD):
            pe_p3[b].append(nc.tensor.matmul(
                cop0[:],
                lhsT=cxT[:, k, :],
                rhs=wc_sb[:, k, 0:512],
                start=(k == 0),
                stop=False,
            ))
            pe_p3[b].append(nc.tensor.matmul(
                cop1[:],
                lhsT=cxT[:, k, :],
                rhs=wc_sb[:, k, 512:H],
                start=(k == 0),
                stop=False,
            ))
        pe_p3[b].append(nc.tensor.matmul(
            cop0[:], lhsT=ones_sb[:, 0:T], rhs=wcb_sb[:, 0:512],
            start=False, stop=True,
        ))
        pe_p3[b].append(nc.tensor.matmul(
            cop1[:], lhsT=ones_sb[:, 0:T], rhs=wcb_sb[:, 512:H],
            start=False, stop=True,
        ))
        coF = sp.tile([T, H], F32, tag="coF")
        nc.vector.tensor_tensor(
            coF[:, 0:512], cop0[:], hid_sb[:, 0:512], op=ALU.mult
        )
        nc.vector.tensor_tensor(
            coF[:, 512:H], cop1[:], hid_sb[:, 512:H], op=ALU.mult
        )
        nc.sync.dma_start(io["co_att"][b], coF[:])

    # PE tiling-mode fences: the scores matmuls run in (128,32) column-tiled
    # mode while everything else is (128,128). Interleaving the modes corrupts
    # in-flight matmuls, so order the PE stream into clean mode groups.
    # Each phase consists of psum accumulation chains whose interiors are
    # already ordered; connecting chain tails to next-phase chain heads with
    # order-only (sync=False) edges keeps the groups contiguous on the PE.
    def _ht_p1(lst):
        assert len(lst) == 96
        heads = [lst[i] for i in (0, 17, 34, 51, 68, 75, 82, 89)]
        tails = [lst[i] for i in (16, 33, 50, 67, 74, 81, 88, 95)]
        return heads, tails

    def _ht_p2(lst):
        assert len(lst) == 256
        return [lst[0]], [lst[-1]]

    def _ht_p3(lst):
        assert len(lst) == 68
        heads = [lst[i] for i in (0, 1, 2, 18, 34, 35)]
        tails = [lst[i] for i in (0, 1, 17, 33, 66, 67)]
        return heads, tails

    if P1_ONLY:
        return
    groups = []
    for b in range(BPC):
        groups.append(_ht_p1(pe_p1[b]))
        groups.append(_ht_p2(pe_p2[b]))
        groups.append(_ht_p3(pe_p3[b]))
    # merge p3[0] and p1[1] into one (both run in 128x128 mode)
    g = [groups[0], groups[1],
         (groups[2][0] + groups[3][0], groups[2][1] + groups[3][1]),
         groups[4], groups[5]]
    for (ph, pt), (qh, qt) in zip(g, g[1:]):
        for h_ in qh:
            for t_ in pt:
                # add_dep_helper(waiter, dependency): head waits on prev tails
                add_dep_helper(h_.ins, t_.ins, sync=False, reason="pe mode fence")


_PROGRAM_CACHE: dict = {}


def _build_program():
    if "nc" in _PROGRAM_CACHE:
        return _PROGRAM_CACHE["nc"]
    nc = bacc.Bacc("TRN2", target_bir_lowering=False, debug=False,
                   num_devices=NCORES)
    io = {
        "maps": nc.dram_tensor("maps", [BPC, SP, D], BF16, kind="ExternalInput").ap(),
        "hiddens": nc.dram_tensor("hiddens", [BPC, T, H], BF16, kind="ExternalInput").ap(),
        "wm": nc.dram_tensor("wm", [D, A], BF16, kind="ExternalInput").ap(),
        "bm": nc.dram_tensor("bm", [1, A], BF16, kind="ExternalInput").ap(),
        "wh": nc.dram_tensor("wh", [H, A], BF16, kind="ExternalInput").ap(),
        "bh": nc.dram_tensor("bh", [1, A], BF16, kind="ExternalInput").ap(),
        "wsgm": nc.dram_tensor("wsgm", [128, NJ, T, T], BF16, kind="ExternalInput").ap(),
        "wc": nc.dram_tensor("wc", [D, H], BF16, kind="ExternalInput").ap(),
        "wcb": nc.dram_tensor("wcb", [1, H], BF16, kind="ExternalInput").ap(),
        "co_att": nc.dram_tensor("co_att", [BPC, T, H], F32, kind="ExternalOutput").ap(),
        "softmap": nc.dram_tensor("softmap", [BPC, T, S], F32, kind="ExternalOutput").ap(),
    }
    if P1_ONLY:
        io["dbg_ml"] = nc.dram_tensor("dbg_ml", [BPC, 128, NJ, S], BF16, kind="ExternalOutput").ap()
        io["dbg_hl"] = nc.dram_tensor("dbg_hl", [BPC, 128, NJ, T], F32, kind="ExternalOutput").ap()
    with tile.TileContext(nc) as tc:
        with ExitStack() as ctx:
            _emit(ctx, tc, io)
    nc.compile()
    _PROGRAM_CACHE["nc"] = nc
    return nc


def _host_prep(inputs: dict) -> list[dict]:
    f32 = lambda x: np.asarray(x, dtype=np.float32)
    maps = f32(inputs["maps"]).reshape(B, S, D)
    hiddens = f32(inputs["hiddens"])
    Wm, bm = f32(inputs["Wm"]), f32(inputs["bm"])
    Wh, bh = f32(inputs["Wh"]), f32(inputs["bh"])
    Wr = f32(inputs["Wr"]).reshape(A)
    Wc, bc = f32(inputs["Wc"]), f32(inputs["bc"])

    absr = np.abs(Wr)
    sgn = np.sign(Wr)
    wm_f = (Wm * absr[None, :]).astype(NPBF16)
    bm_f = (bm * absr).reshape(1, A).astype(NPBF16)
    wh_f = (Wh * absr[None, :]).astype(NPBF16)
    bh_f = (bh * absr).reshape(1, A).astype(NPBF16)
    sgn_pj = sgn.reshape(NJ, 128).T  # [p, j] = sign(Wr[128j+p])
    wsgm = np.zeros((128, NJ, T, T), np.float32)
    for t in range(T):
        wsgm[:, :, t, t] = sgn_pj
    wsgm = wsgm.astype(NPBF16)
    wc_f = Wc.astype(NPBF16)
    wcb = bc.reshape(1, H).astype(NPBF16)
    maps_pad = np.zeros((B, SP, D), np.float32)
    maps_pad[:, :S, :] = maps
    maps_bf = maps_pad.astype(NPBF16)
    hid_bf = hiddens.astype(NPBF16)

    in_maps = []
    for i in range(NCORES):
        in_maps.append({
            "maps": maps_bf[BPC * i : BPC * (i + 1)],
            "hiddens": hid_bf[BPC * i : BPC * (i + 1)],
            "wm": wm_f, "bm": bm_f, "wh": wh_f, "bh": bh_f,
            "wsgm": wsgm, "wc": wc_f, "wcb": wcb,
        })
    return in_maps


def kernel(**inputs):
    nc = _build_program()
    in_maps = _host_prep(inputs)
    res = run_bass_kernel_spmd(nc, in_maps, list(range(NCORES)))
    co = np.concatenate([r["co_att"] for r in res.results], axis=0)
    sm = np.concatenate([r["softmap"] for r in res.results], axis=0)
    return co.astype(np.float32), sm.astype(np.float32)


if __name__ == "__main__":
    rng = np.random.default_rng(0)
    fake = {
        "maps": rng.standard_normal((B, 14, 14, D), dtype=np.float32),
        "hiddens": rng.standard_normal((B, T, H), dtype=np.float32),
        "Wm": rng.standard_normal((D, A), dtype=np.float32) / np.sqrt(D),
        "bm": np.zeros(A, np.float32),
        "Wh": rng.standard_normal((H, A), dtype=np.float32) / np.sqrt(H),
        "bh": np.zeros(A, np.float32),
        "Wr": rng.standard_normal(A, dtype=np.float32) / np.sqrt(A),
        "br": np.zeros((), np.float32),
        "Wc": rng.standard_normal((D, H), dtype=np.float32) / np.sqrt(D),
        "bc": np.zeros(H, np.float32),
    }
    co, sm = kernel(**fake)
    print(co.shape, sm.shape, co.dtype, sm.dtype)


# revision 20
# speedup vs baseline: 1.0697x; 1.0697x over previous
"""CoAttention kernel for 8 TRN2 NeuronCores (data-parallel over batch).

Math (per batch b):
  fm = maps.reshape(B, 196, 2048)
  ml = fm @ Wm + bm                       (196, 512)
  hl = hiddens @ Wh + bh                  (64, 512)
  scores[t,s] = sum_a Wr[a]*relu(ml[s,a]+hl[t,a]) + br
  softmap = softmax(scores, axis=s)
  ctx = softmap @ fm                      (64, 2048)
  co_att = (ctx @ Wc + bc) * hiddens      (64, 768)
  returns (co_att, softmap)

Key tricks:
  * br cancels in the softmax -> dropped entirely.
  * |Wr| is folded into Wm/bm/Wh/bh host-side, so
    Wr[a]*relu(ml+hl) == sign(Wr[a]) * relu(ml''+hl'') elementwise, and the
    a-reduction becomes a single ones/sign-weighted matmul (lhsT = sign vec).
  * rect = relu(ml'' + hl''_t) is one fused DVE tensor_scalar (add, max 0)
    or one ACT activation(Relu, bias) per (t, a-chunk); both engines share it.
  * The a-reduction runs on the PE with 128x32 column tiling: 4 independent
    column tiles each own 16 of the 64 t's (4x matmul throughput at M=1).
  * bm/bh/bc are folded in as K=1 rank-1 bias matmuls (ones x bias_row).
"""

import os
import sys

for _p in ("/opt/trn_rl_repo",):
    if _p not in sys.path:
        sys.path.insert(0, _p)

from contextlib import ExitStack

import ml_dtypes
import numpy as np

import concourse.bass as bass
import concourse.mybir as mybir
import concourse.tile as tile
from concourse import bacc
from concourse.bass_utils import run_bass_kernel_spmd
from concourse.masks import make_identity
from concourse.tile import add_dep_helper

BF16 = mybir.dt.bfloat16
F32 = mybir.dt.float32
AF = mybir.ActivationFunctionType
ALU = mybir.AluOpType
NPBF16 = ml_dtypes.bfloat16

B, S, D, H, A, T = 16, 196, 2048, 768, 512, 64
NCORES = 8
BPC = B // NCORES  # batches per core
NJ = A // 128      # 4 a-chunks
KD = D // 128      # 16
KH = H // 128      # 6
SC = S // 2        # 98, s-halves for ctx contraction
SP = 208           # maps rows padded to a multiple of 16 for the xbar transpose

USE_COL_TILING = True
P1_ONLY = bool(os.environ.get("K_P1_ONLY"))
ACT_RECT_FRAC = 4   # every 4th t's rect ops go to ScalarE, rest on VectorE


def _emit(ctx: ExitStack, tc: "tile.TileContext", io: dict):
    nc = tc.nc
    wp = ctx.enter_context(tc.tile_pool(name="wts", bufs=1))
    dp = ctx.enter_context(tc.tile_pool(name="data", bufs=2))
    mp = ctx.enter_context(tc.tile_pool(name="mlhl", bufs=2))
    rp = ctx.enter_context(tc.tile_pool(name="rect", bufs=8))
    sp = ctx.enter_context(tc.tile_pool(name="small", bufs=2))
    pm = ctx.enter_context(tc.tile_pool(name="psP1", bufs=1, space="PSUM"))
    psc = ctx.enter_context(tc.tile_pool(name="psSc", bufs=1, space="PSUM"))
    p3 = ctx.enter_context(tc.tile_pool(name="psP3", bufs=1, space="PSUM"))

    # ---- weights / constants (loaded once) ----
    wm_sb = wp.tile([128, KD, A], BF16)
    nc.sync.dma_start(wm_sb[:], io["wm"].rearrange("(c p) a -> p c a", p=128))
    wh_sb = wp.tile([128, KH, A], BF16)
    nc.sync.dma_start(wh_sb[:], io["wh"].rearrange("(c p) a -> p c a", p=128))
    wc_sb = wp.tile([128, KD, H], BF16)
    nc.sync.dma_start(wc_sb[:], io["wc"].rearrange("(c p) h -> p c h", p=128))
    bm_sb = wp.tile([1, A], BF16)
    nc.sync.dma_start(bm_sb[:], io["bm"][:])
    bh_sb = wp.tile([1, A], BF16)
    nc.sync.dma_start(bh_sb[:], io["bh"][:])
    wcb_sb = wp.tile([1, H], BF16)
    nc.sync.dma_start(wcb_sb[:], io["wcb"][:])
    # wsgm[p, j, i, m] = sign(Wr[128j+p]) if m == i else 0 — one-hot sign
    # columns so column tile c can write t=16c+i into psum row 32c+i of its
    # own scores bank while each matmul's output starts at quadrant base 32c.
    wsgm_sb = wp.tile([128, NJ, 16, 16], BF16)
    nc.sync.dma_start(wsgm_sb[:], io["wsgm"][:])
    ones_sb = wp.tile([1, S], BF16)
    nc.vector.memset(ones_sb[:], 1.0)
    ident = wp.tile([128, 128], BF16)
    make_identity(nc, ident[:])

    pe_p1 = [[] for _ in range(BPC)]
    pe_p2 = [[] for _ in range(BPC)]
    pe_p3 = [[] for _ in range(BPC)]

    for b in range(BPC):
        # ---- P1: load batch data, compute ml'' and hl'' ----
        fmT = dp.tile([128, KD, SP], BF16, tag="fmT")
        for c in range(KD):
            # xbar transpose needs src rows % 16 == 0, hence the host-side
            # zero-pad of maps to SP=208 rows (the 4-row remainder fallback
            # produces garbage on hardware)
            nc.sync.dma_start_transpose(
                fmT[:, c, :], io["maps"][b, :, 128 * c : 128 * (c + 1)]
            )
        hidT = dp.tile([128, KH, T], BF16, tag="hidT")
        for c in range(KH):
            nc.sync.dma_start_transpose(
                hidT[:, c, :], io["hiddens"][b, :, 128 * c : 128 * (c + 1)]
            )
        fm_sb = dp.tile([SC, 2, D], BF16, tag="fm")
        nc.sync.dma_start(fm_sb[:, 0, :], io["maps"][b, 0:SC, :])
        nc.sync.dma_start(fm_sb[:, 1, :], io["maps"][b, SC:S, :])
        hid_sb = dp.tile([T, H], BF16, tag="hid")
        nc.sync.dma_start(hid_sb[:], io["hiddens"][b])

        # ml'' = (fm @ Wm'' + bm'')^T : NJ chunks of (128a, S), psum packed in pairs
        ml_sb = mp.tile([128, NJ, S], BF16, tag="ml")
        for jj in range(2):
            mlp = pm.tile([128, 2, S], F32, tag="p1ps")
            for j2 in range(2):
                j = 2 * jj + j2
                for kc in range(KD):
                    pe_p1[b].append(nc.tensor.matmul(
                        mlp[:, j2, :],
                        lhsT=wm_sb[:, kc, 128 * j : 128 * (j + 1)],
                        rhs=fmT[:, kc, 0:S],
                        start=(kc == 0),
                        stop=False,
                    ))
                pe_p1[b].append(nc.tensor.matmul(
                    mlp[:, j2, :],
                    lhsT=bm_sb[:, 128 * j : 128 * (j + 1)],
                    rhs=ones_sb[:],
                    start=False,
                    stop=True,
                ))
            nc.vector.tensor_copy(ml_sb[:, 2 * jj : 2 * jj + 2, :], mlp[:])

        # hl'' = (hiddens @ Wh'' + bh'')^T : (128a, NJ, T) in one psum bank
        hlp = pm.tile([128, NJ, T], F32, tag="p1ps")
        for j in range(NJ):
            for kc in range(KH):
                pe_p1[b].append(nc.tensor.matmul(
                    hlp[:, j, :],
                    lhsT=wh_sb[:, kc, 128 * j : 128 * (j + 1)],
                    rhs=hidT[:, kc, :],
                    start=(kc == 0),
                    stop=False,
                ))
            pe_p1[b].append(nc.tensor.matmul(
                hlp[:, j, :],
                lhsT=bh_sb[:, 128 * j : 128 * (j + 1)],
                rhs=ones_sb[:, 0:T],
                start=False,
                stop=True,
            ))
        hl_sb = mp.tile([128, NJ, T], F32, tag="hl")
        nc.vector.tensor_copy(hl_sb[:], hlp[:])

        if P1_ONLY:
            nc.sync.dma_start(io["dbg_ml"][b], ml_sb[:])
            nc.sync.dma_start(io["dbg_hl"][b], hl_sb[:])
            zf = sp.tile([T, H], F32, tag="coF")
            nc.vector.memset(zf[:], 0.0)
            nc.sync.dma_start(io["co_att"][b], zf[:])
            zs = sp.tile([T, S], F32, tag="smF")
            nc.vector.memset(zs[:], 0.0)
            nc.sync.dma_start(io["softmap"][b], zs[:])
            continue

        # ---- P2: rect + sign-weighted a-reduction -> scores ----
        # 4 independent column tiles (128x32 mode), each with its OWN psum
        # bank so the four accumulation groups never share a bank; tile c
        # owns t in [16c, 16c+16) and writes rows [32c, 32c+16) of its bank.
        scq = [psc.tile([128, 256], F32, tag=f"sc{c}", name=f"scq{c}") for c in range(4)]
        for i in range(16):
            for c in range(4):
                t = 16 * c + i
                rect = rp.tile([128, NJ, S], BF16, tag="rect")
                on_act = (t % ACT_RECT_FRAC) == ACT_RECT_FRAC - 1
                for j in range(NJ):
                    if on_act:
                        nc.scalar.activation(
                            rect[:, j, :],
                            ml_sb[:, j, :],
                            AF.Relu,
                            bias=hl_sb[:, j, t : t + 1],
                        )
                    else:
                        nc.vector.tensor_scalar(
                            rect[:, j, :],
                            ml_sb[:, j, :],
                            hl_sb[:, j, t : t + 1],
                            0.0,
                            op0=ALU.add,
                            op1=ALU.max,
                        )
                for j in range(NJ):
                    pe_p2[b].append(nc.tensor.matmul(
                        scq[c][32 * c : 32 * c + 16, 0:S],
                        lhsT=wsgm_sb[:, j, i, :],
                        rhs=rect[:, j, :],
                        start=(i == 0 and j == 0),
                        stop=(i == 15 and j == NJ - 1),
                        skip_group_check=True,
                        tile_position=(0, 32 * c),
                    ))

        # ---- P3: softmax, softmap out, ctx, co_att ----
        exps = sp.tile([128, S], BF16, tag="exps")
        for c in range(4):
            nc.scalar.activation(
                exps[32 * c : 32 * c + 16, :],
                scq[c][32 * c : 32 * c + 16, 0:S],
                AF.Exp,
            )
        den = sp.tile([128, 1], F32, tag="den")
        nc.vector.reduce_sum(den[:], exps[:], axis=mybir.AxisListType.X)
        rec = sp.tile([128, 1], F32, tag="rec")
        nc.vector.reciprocal(rec[:], den[:])
        smF = sp.tile([128, S], F32, tag="smF")
        nc.vector.tensor_scalar(
            smF[:], exps[:], rec[:, 0:1], None, op0=ALU.mult
        )
        for c in range(4):
            nc.sync.dma_start(
                io["softmap"][b, 16 * c : 16 * (c + 1), :],
                smF[32 * c : 32 * c + 16, :],
            )
        smN = sp.tile([128, S], BF16, tag="smN")
        nc.vector.tensor_scalar(
            smN[:], exps[:], rec[:, 0:1], None, op0=ALU.mult
        )

        # softmap^T (normalized, bf16) via two full-width PE transposes
        # (K=128 keeps the PE in plain 128x128 mode); valid t-columns sit at
        # the spread positions {32c+i} and are gathered in the psum->sbuf copy.
        smT_ps = p3.tile([SC, 2, 128], BF16, tag="p3a")
        for k in range(2):
            pe_p3[b].append(nc.tensor.transpose(
                smT_ps[:, k, :],
                smN[:, SC * k : SC * (k + 1)],
                ident[:],
            ))
        smT = sp.tile([SC, 2, T], BF16, tag="smT")
        nc.vector.tensor_copy(
            smT[:].rearrange("p k (g q) -> p k g q", q=16),
            smT_ps[:].rearrange("p k (g q) -> p k g q", q=32)[:, :, :, 0:16],
        )

        # ctx^T: (d, t) in two psum banks of 8 d-chunks each
        cxT = sp.tile([128, KD, T], BF16, tag="cxT")
        for hh in range(2):
            cxp = p3.tile([128, 8, T], F32, tag="p3a")
            for k8 in range(8):
                k = 8 * hh + k8
                for k2 in range(2):
                    pe_p3[b].append(nc.tensor.matmul(
                        cxp[:, k8, :],
                        lhsT=fm_sb[:, k2, 128 * k : 128 * (k + 1)],
                        rhs=smT[:, k2, :],
                        start=(k2 == 0),
                        stop=(k2 == 1),
                    ))
            nc.vector.tensor_copy(cxT[:, 8 * hh : 8 * hh + 8, :], cxp[:])

        # co_att = (ctx @ Wc + bc) * hiddens
        cop0 = p3.tile([T, 512], F32, tag="cop0")
        cop1 = p3.tile([T, H - 512], F32, tag="cop1")
        for k in range(KD):
            pe_p3[b].append(nc.tensor.matmul(
                cop0[:],
                lhsT=cxT[:, k, :],
                rhs=wc_sb[:, k, 0:512],
                start=(k == 0),
                stop=False,
            ))
            pe_p3[b].append(nc.tensor.matmul(
                cop1[:],
                lhsT=cxT[:, k, :],
                rhs=wc_sb[:, k, 512:H],
                start=(k == 0),
                stop=False,
            ))
        pe_p3[b].append(nc.tensor.matmul(
            cop0[:], lhsT=ones_sb[:, 0:T], rhs=wcb_sb[:, 0:512],
            start=False, stop=True,
        ))
        pe_p3[b].append(nc.tensor.matmul(
            cop1[:], lhsT=ones_sb[:, 0:T], rhs=wcb_sb[:, 512:H],
            start=False, stop=True,
        ))
        coF = sp.tile([T, H], F32, tag="coF")
        nc.vector.tensor_tensor(
            coF[:, 0:512], cop0[:], hid_sb[:, 0:512], op=ALU.mult
        )
        nc.vector.tensor_tensor(
            coF[:, 512:H], cop1[:], hid_sb[:, 512:H], op=ALU.mult
        )
        nc.sync.dma_start(io["co_att"][b], coF[:])

    # PE tiling-mode fences: the scores matmuls run in (128,32) column-tiled
    # mode while everything else is (128,128). Interleaving the modes corrupts
    # in-flight matmuls, so order the PE stream into clean mode groups.
    # Each phase consists of psum accumulation chains whose interiors are
    # already ordered; connecting chain tails to next-phase chain heads with
    # order-only (sync=False) edges keeps the groups contiguous on the PE.
    def _ht_p1(lst):
        assert len(lst) == 96
        heads = [lst[i] for i in (0, 17, 34, 51, 68, 75, 82, 89)]
        tails = [lst[i] for i in (16, 33, 50, 67, 74, 81, 88, 95)]
        return heads, tails

    def _ht_p2(lst):
        assert len(lst) == 256
        heads = [lst[4 * c] for c in range(4)]
        tails = [lst[(60 + c) * 4 + 3] for c in range(4)]
        return heads, tails

    def _ht_p3(lst):
        assert len(lst) == 68
        heads = [lst[i] for i in (0, 1, 2, 18, 34, 35)]
        tails = [lst[i] for i in (0, 1, 17, 33, 66, 67)]
        return heads, tails

    if P1_ONLY:
        return
    groups = []
    for b in range(BPC):
        groups.append(_ht_p1(pe_p1[b]))
        groups.append(_ht_p2(pe_p2[b]))
        groups.append(_ht_p3(pe_p3[b]))
    # merge p3[0] and p1[1] into one (both run in 128x128 mode)
    g = [groups[0], groups[1],
         (groups[2][0] + groups[3][0], groups[2][1] + groups[3][1]),
         groups[4], groups[5]]
    for (ph, pt), (qh, qt) in zip(g, g[1:]):
        for h_ in qh:
            for t_ in pt:
                # add_dep_helper(waiter, dependency): head waits on prev tails
                add_dep_helper(h_.ins, t_.ins, sync=False, reason="pe mode fence")


_PROGRAM_CACHE: dict = {}


def _build_program():
    if "nc" in _PROGRAM_CACHE:
        return _PROGRAM_CACHE["nc"]
    nc = bacc.Bacc("TRN2", target_bir_lowering=False, debug=False,
                   num_devices=NCORES)
    io = {
        "maps": nc.dram_tensor("maps", [BPC, SP, D], BF16, kind="ExternalInput").ap(),
        "hiddens": nc.dram_tensor("hiddens", [BPC, T, H], BF16, kind="ExternalInput").ap(),
        "wm": nc.dram_tensor("wm", [D, A], BF16, kind="ExternalInput").ap(),
        "bm": nc.dram_tensor("bm", [1, A], BF16, kind="ExternalInput").ap(),
        "wh": nc.dram_tensor("wh", [H, A], BF16, kind="ExternalInput").ap(),
        "bh": nc.dram_tensor("bh", [1, A], BF16, kind="ExternalInput").ap(),
        "wsgm": nc.dram_tensor("wsgm", [128, NJ, 16, 16], BF16, kind="ExternalInput").ap(),
        "wc": nc.dram_tensor("wc", [D, H], BF16, kind="ExternalInput").ap(),
        "wcb": nc.dram_tensor("wcb", [1, H], BF16, kind="ExternalInput").ap(),
        "co_att": nc.dram_tensor("co_att", [BPC, T, H], F32, kind="ExternalOutput").ap(),
        "softmap": nc.dram_tensor("softmap", [BPC, T, S], F32, kind="ExternalOutput").ap(),
    }
    if P1_ONLY:
        io["dbg_ml"] = nc.dram_tensor("dbg_ml", [BPC, 128, NJ, S], BF16, kind="ExternalOutput").ap()
        io["dbg_hl"] = nc.dram_tensor("dbg_hl", [BPC, 128, NJ, T], F32, kind="ExternalOutput").ap()
    with tile.TileContext(nc) as tc:
        with ExitStack() as ctx:
            _emit(ctx, tc, io)
    nc.compile()
    _PROGRAM_CACHE["nc"] = nc
    return nc


def _host_prep(inputs: dict) -> list[dict]:
    f32 = lambda x: np.asarray(x, dtype=np.float32)
    maps = f32(inputs["maps"]).reshape(B, S, D)
    hiddens = f32(inputs["hiddens"])
    Wm, bm = f32(inputs["Wm"]), f32(inputs["bm"])
    Wh, bh = f32(inputs["Wh"]), f32(inputs["bh"])
    Wr = f32(inputs["Wr"]).reshape(A)
    Wc, bc = f32(inputs["Wc"]), f32(inputs["bc"])

    absr = np.abs(Wr)
    sgn = np.sign(Wr)
    wm_f = (Wm * absr[None, :]).astype(NPBF16)
    bm_f = (bm * absr).reshape(1, A).astype(NPBF16)
    wh_f = (Wh * absr[None, :]).astype(NPBF16)
    bh_f = (bh * absr).reshape(1, A).astype(NPBF16)
    sgn_pj = sgn.reshape(NJ, 128).T  # [p, j] = sign(Wr[128j+p])
    wsgm = np.zeros((128, NJ, 16, 16), np.float32)
    for i in range(16):
        wsgm[:, :, i, i] = sgn_pj
    wsgm = wsgm.astype(NPBF16)
    wc_f = Wc.astype(NPBF16)
    wcb = bc.reshape(1, H).astype(NPBF16)
    maps_pad = np.zeros((B, SP, D), np.float32)
    maps_pad[:, :S, :] = maps
    maps_bf = maps_pad.astype(NPBF16)
    hid_bf = hiddens.astype(NPBF16)

    in_maps = []
    for i in range(NCORES):
        in_maps.append({
            "maps": maps_bf[BPC * i : BPC * (i + 1)],
            "hiddens": hid_bf[BPC * i : BPC * (i + 1)],
            "wm": wm_f, "bm": bm_f, "wh": wh_f, "bh": bh_f,
            "wsgm": wsgm, "wc": wc_f, "wcb": wcb,
        })
    return in_maps


def kernel(**inputs):
    nc = _build_program()
    in_maps = _host_prep(inputs)
    res = run_bass_kernel_spmd(nc, in_maps, list(range(NCORES)))
    co = np.concatenate([r["co_att"] for r in res.results], axis=0)
    sm = np.concatenate([r["softmap"] for r in res.results], axis=0)
    return co.astype(np.float32), sm.astype(np.float32)


if __name__ == "__main__":
    rng = np.random.default_rng(0)
    fake = {
        "maps": rng.standard_normal((B, 14, 14, D), dtype=np.float32),
        "hiddens": rng.standard_normal((B, T, H), dtype=np.float32),
        "Wm": rng.standard_normal((D, A), dtype=np.float32) / np.sqrt(D),
        "bm": np.zeros(A, np.float32),
        "Wh": rng.standard_normal((H, A), dtype=np.float32) / np.sqrt(H),
        "bh": np.zeros(A, np.float32),
        "Wr": rng.standard_normal(A, dtype=np.float32) / np.sqrt(A),
        "br": np.zeros((), np.float32),
        "Wc": rng.standard_normal((D, H), dtype=np.float32) / np.sqrt(D),
        "bc": np.zeros(H, np.float32),
    }
    co, sm = kernel(**fake)
    print(co.shape, sm.shape, co.dtype, sm.dtype)
